# revision 5
# baseline (speedup 1.0000x reference)
"""Trainium2 Bass kernel for nn_DAriEL_Decoder_Cell_1_88064009437441.

Key structural fact about the reference: the decoder cell resets
`one_softmax`/`unfolding` to their initial values at every t>0 (faithful
tf.cond port), so token selection at step t uses the UNIFORM distribution
and input_point[:, t] only — tokens never depend on the LM. The LM outputs
(the actual kernel result) are softmax(h_t) of a single 8-step LSTM scan
over the decoded tokens, since the per-step prefixes are nested.

Host: exact uniform-interval token search (all quantities are dyadic
rationals, exact in fp32) + embedding gather + weight re-layout.
Device (8 cores, SPMD): gate-dim (hidden-unit) sharded LSTM, 256 units
per core; per step each core computes its z-block = [h; x_t; 1] @ Waug_blk
via PE matmuls (batch=16 output partitions, N=512 moving), applies gate
activations, updates (h,c) slice, PE-transposes its h slice and AllGathers
[hT_slice | exp-partial-sums] so every core has the full transposed h for
the next step's matmul. Softmax is assembled per-core over its own slice
with the gathered global sums (no max-subtraction needed: |h|<1).
"""

import numpy as np

VOCAB = 2048
EMB = 256
MAXLEN = 8
BATCH = 16
NCORES = 8
U = VOCAB // NCORES          # 256 hidden units per core
KT = 18                      # contraction k-tiles: 16 (h) + 2 (x)
AGW = 257 * 16               # allgather payload floats per core (4112)

_CACHE = {}


def _host_tokens(input_point):
    """token[b,t] = argmax_k((k/V <= v) & (v <= (k+1)/V)), first-true wins.
    Exact: v is fp32, k/V dyadic; replicate in float64."""
    v = input_point[:, :MAXLEN].astype(np.float64)
    u = v * VOCAB
    j = np.floor(u)
    exact = (u == j) & (j > 0)
    tok = np.where(exact, j - 1, j)
    return np.clip(tok, 0, VOCAB - 1).astype(np.int32)


def _build_nc():
    import concourse.bass as bass
    import concourse.mybir as mybir
    import concourse.tile as tile
    from concourse import bacc
    from concourse.masks import make_identity

    f32 = mybir.dt.float32
    AF = mybir.ActivationFunctionType
    OP = mybir.AluOpType

    nc = bacc.Bacc(None, target_bir_lowering=False, debug=False)

    wblk = nc.dram_tensor("wblk", [2305, 4 * U], f32, kind="ExternalInput")
    xt_ext = nc.dram_tensor("xt", [128, 2 * MAXLEN * BATCH], f32, kind="ExternalInput")
    mask_ext = nc.dram_tensor("masks", [BATCH, MAXLEN], f32, kind="ExternalInput")
    out_ext = nc.dram_tensor("out", [BATCH, MAXLEN * U], f32, kind="ExternalOutput")

    rg = [list(range(NCORES))]

    with tile.TileContext(nc) as tc:
        with (
            tc.tile_pool(name="const", bufs=1) as cp,
            tc.tile_pool(name="state", bufs=2) as sp,
            tc.tile_pool(name="work", bufs=3) as wk,
            tc.tile_pool(name="zps", bufs=2, space="PSUM") as psz,
            tc.tile_pool(name="trps", bufs=2, space="PSUM") as pst,
            tc.tile_pool(name="dram", bufs=3, space="DRAM") as dp,
        ):
            identity = cp.tile([128, 128], f32)
            make_identity(nc, identity[:])
            ones16 = cp.tile([1, 16], f32)
            nc.gpsimd.memset(ones16[:], 1.0)

            xt_sb = cp.tile([128, 2 * MAXLEN * BATCH], f32)
            nc.sync.dma_start(xt_sb[:], xt_ext[:])
            mask_sb = cp.tile([BATCH, MAXLEN], f32)
            nc.sync.dma_start(mask_sb[:], mask_ext[:])

            wsb = cp.tile([128, KT * 1024], f32)
            wb = cp.tile([1, 1024], f32)
            # x-tiles + bias first (step 0 needs only those), then h-tiles
            for j in (16, 17):
                nc.sync.dma_start(
                    wsb[:, 1024 * j:1024 * (j + 1)], wblk[128 * j:128 * (j + 1), :]
                )
            nc.sync.dma_start(wb[:], wblk[2304:2305, :])
            for j in range(16):
                nc.sync.dma_start(
                    wsb[:, 1024 * j:1024 * (j + 1)], wblk[128 * j:128 * (j + 1), :]
                )

            exp_sb = cp.tile([BATCH, MAXLEN * U], f32)
            out_sb = cp.tile([BATCH, MAXLEN * U], f32)

            h_prev = sp.tile([BATCH, U], f32, tag="h")
            c_prev = sp.tile([BATCH, U], f32, tag="c")
            nc.vector.memset(h_prev[:], 0.0)
            nc.vector.memset(c_prev[:], 0.0)
            hT_full = None

            for t in range(MAXLEN):
                # ---- z = [x_t; h; 1] @ Waug_blk  (two 512-col PSUM banks) ----
                pz0 = psz.tile([BATCH, 512], f32, tag="z0")
                pz1 = psz.tile([BATCH, 512], f32, tag="z1")
                for pz, cb in ((pz0, 0), (pz1, 512)):
                    nc.tensor.matmul(
                        pz[:], xt_sb[:, 16 * t:16 * t + 16],
                        wsb[:, 1024 * 16 + cb:1024 * 16 + cb + 512],
                        start=True, stop=False,
                    )
                    nc.tensor.matmul(
                        pz[:], xt_sb[:, 128 + 16 * t:128 + 16 * t + 16],
                        wsb[:, 1024 * 17 + cb:1024 * 17 + cb + 512],
                        start=False, stop=False,
                    )
                    if t > 0:
                        for j in range(16):
                            nc.tensor.matmul(
                                pz[:], hT_full[:, 16 * j:16 * j + 16],
                                wsb[:, 1024 * j + cb:1024 * j + cb + 512],
                                start=False, stop=False,
                            )
                    nc.tensor.matmul(
                        pz[:], ones16[:], wb[:, cb:cb + 512],
                        start=False, stop=True,
                    )

                # ---- gates: pz0 = [i|f], pz1 = [g|o] ----
                si = wk.tile([BATCH, U], f32, tag="si")
                sf = wk.tile([BATCH, U], f32, tag="sf")
                tg = wk.tile([BATCH, U], f32, tag="tg")
                so = wk.tile([BATCH, U], f32, tag="so")
                nc.scalar.activation(si[:], pz0[:, 0:U], AF.Sigmoid)
                nc.scalar.activation(sf[:], pz0[:, U:2 * U], AF.Sigmoid)
                nc.scalar.activation(tg[:], pz1[:, 0:U], AF.Tanh)
                nc.scalar.activation(so[:], pz1[:, U:2 * U], AF.Sigmoid)

                m_t = mask_sb[:, t:t + 1]

                # ---- c update (with PAD mask) ----
                t1 = wk.tile([BATCH, U], f32, tag="t1")
                t2 = wk.tile([BATCH, U], f32, tag="t2")
                cn = wk.tile([BATCH, U], f32, tag="cn")
                dm = wk.tile([BATCH, U], f32, tag="dm")
                c_next = sp.tile([BATCH, U], f32, tag="c")
                nc.vector.tensor_tensor(t1[:], sf[:], c_prev[:], OP.mult)
                nc.vector.tensor_tensor(t2[:], si[:], tg[:], OP.mult)
                nc.vector.tensor_tensor(cn[:], t1[:], t2[:], OP.add)
                nc.vector.tensor_tensor(dm[:], cn[:], c_prev[:], OP.subtract)
                nc.vector.scalar_tensor_tensor(
                    c_next[:], dm[:], m_t, c_prev[:], OP.mult, OP.add
                )

                # ---- h update ----
                tcx = wk.tile([BATCH, U], f32, tag="tcx")
                hn = wk.tile([BATCH, U], f32, tag="hn")
                dh = wk.tile([BATCH, U], f32, tag="dh")
                h_next = sp.tile([BATCH, U], f32, tag="h")
                nc.scalar.activation(tcx[:], c_next[:], AF.Tanh)
                nc.vector.tensor_tensor(hn[:], so[:], tcx[:], OP.mult)
                nc.vector.tensor_tensor(dh[:], hn[:], h_prev[:], OP.subtract)
                nc.vector.scalar_tensor_tensor(
                    h_next[:], dh[:], m_t, h_prev[:], OP.mult, OP.add
                )

                # ---- exp(h) for softmax numerator + per-shard partial sum ----
                acc = wk.tile([BATCH, 1], f32, tag="acc")
                nc.scalar.activation(
                    exp_sb[:, U * t:U * (t + 1)], h_next[:], AF.Exp,
                    accum_out=acc[:],
                )

                # ---- transpose own h slice (16,256) -> (256,16) via PE ----
                ptr0 = pst.tile([128, 16], f32, tag="tr0")
                ptr1 = pst.tile([128, 16], f32, tag="tr1")
                nc.tensor.transpose(ptr0[:], h_next[:, 0:128], identity[0:16, 0:16])
                nc.tensor.transpose(ptr1[:], h_next[:, 128:256], identity[0:16, 0:16])
                trs = wk.tile([128, 32], f32, tag="trs")
                nc.vector.tensor_copy(trs[:, 0:16], ptr0[:])
                nc.vector.tensor_copy(trs[:, 16:32], ptr1[:])

                # ---- AllGather [hT_slice (256x16) | partial sums (16)] ----
                agin = dp.tile([257, 16], f32, tag="agin")
                agout = dp.tile([NCORES, AGW], f32, tag="agout")
                nc.sync.dma_start(agin[0:128, :], trs[:, 0:16])
                nc.sync.dma_start(agin[128:256, :], trs[:, 16:32])
                nc.sync.dma_start(
                    agin[256:257, :].rearrange("a b -> b a"), acc[:]
                )
                nc.gpsimd.collective_compute(
                    "AllGather",
                    OP.bypass,
                    replica_groups=rg,
                    ins=[agin[:, :].opt()],
                    outs=[agout[:, :].opt()],
                )

                # ---- unpack gathered hT (not needed after last step) ----
                if t < MAXLEN - 1:
                    hT_full = sp.tile([128, 16 * 16], f32, tag="hT")
                    for m in (0, 1):
                        nc.sync.dma_start(
                            hT_full[:].rearrange("p (r q) -> p r q", q=32)[
                                :, :, 16 * m:16 * m + 16
                            ],
                            agout[:, 2048 * m:2048 * m + 2048].rearrange(
                                "r (p b) -> p r b", p=128, b=16
                            ),
                        )

                # ---- global softmax denominator + own-slice scaling ----
                sums_t = wk.tile([BATCH, NCORES], f32, tag="sums")
                nc.sync.dma_start(
                    sums_t[:], agout[:, 4096:4112].rearrange("r b -> b r")
                )
                tot = wk.tile([BATCH, 1], f32, tag="tot")
                rec = wk.tile([BATCH, 1], f32, tag="rec")
                nc.vector.tensor_reduce(
                    tot[:], sums_t[:], mybir.AxisListType.X, OP.add
                )
                nc.vector.reciprocal(rec[:], tot[:])
                nc.vector.tensor_scalar_mul(
                    out_sb[:, U * t:U * (t + 1)], exp_sb[:, U * t:U * (t + 1)],
                    rec[:],
                )
                nc.sync.dma_start(
                    out_ext[:, U * t:U * (t + 1)], out_sb[:, U * t:U * (t + 1)]
                )

                h_prev, c_prev = h_next, c_next

    nc.compile()
    return nc


def _get_nc():
    if "nc" not in _CACHE:
        _CACHE["nc"] = _build_nc()
    return _CACHE["nc"]


def _host_prep(input_point, E, Wk, Wr, b):
    ip = np.ascontiguousarray(np.asarray(input_point, dtype=np.float32))
    E = np.asarray(E, dtype=np.float32)
    Wk = np.asarray(Wk, dtype=np.float32)
    Wr = np.asarray(Wr, dtype=np.float32)
    b = np.asarray(b, dtype=np.float32)

    tokens = _host_tokens(ip)                                # (B, T)
    masks = (tokens != 0).astype(np.float32)                 # (B, T)
    X = E[tokens]                                            # (B, T, EMB)

    # xt[p, 128*half + 16*t + b] = X[b, t, 128*half + p]
    xt = np.transpose(X.reshape(BATCH, MAXLEN, 2, 128), (2, 3, 1, 0))  # (2,128,T,B)
    xt = np.ascontiguousarray(
        np.transpose(xt, (1, 0, 2, 3)).reshape(128, 2 * MAXLEN * BATCH)
    )

    W_aug = np.vstack([Wr, Wk, b[None, :]]).astype(np.float32)  # (2305, 4V)
    in_maps = []
    for k in range(NCORES):
        cols = np.concatenate(
            [np.arange(g * VOCAB + k * U, g * VOCAB + (k + 1) * U) for g in range(4)]
        )
        in_maps.append({
            "wblk": np.ascontiguousarray(W_aug[:, cols]),
            "xt": xt,
            "masks": np.ascontiguousarray(masks),
        })
    return in_maps


def kernel(input_point, E, Wk, Wr, b):
    from concourse.bass_utils import run_bass_kernel_spmd

    in_maps = _host_prep(input_point, E, Wk, Wr, b)
    nc = _get_nc()
    res = run_bass_kernel_spmd(nc, in_maps, list(range(NCORES)))
    results = res.results

    out = np.empty((BATCH, MAXLEN, VOCAB), dtype=np.float32)
    for k in range(NCORES):
        blk = results[k]["out"].reshape(BATCH, MAXLEN, U)    # (B, T, U)
        out[:, :, k * U:(k + 1) * U] = blk
    return out


# revision 8
# speedup vs baseline: 1.5500x; 1.5500x over previous
"""Trainium2 Bass kernel for nn_DAriEL_Decoder_Cell_1_88064009437441.

Key structural fact about the reference: the decoder cell resets
`one_softmax`/`unfolding` to their initial values at every t>0 (faithful
tf.cond port), so token selection at step t uses the UNIFORM distribution
and input_point[:, t] only — tokens never depend on the LM. The LM outputs
(the actual kernel result) are softmax(h_t) of a single 8-step LSTM scan
over the decoded tokens, since the per-step prefixes are nested.

Host: exact uniform-interval token search (all quantities are dyadic
rationals, exact in fp32) + embedding gather + weight re-layout (bf16).
Device (8 cores, SPMD): gate-dim (hidden-unit) sharded LSTM, 256 units
per core. x@Wk for all 8 steps is computed once up front; per step each
core seeds a PSUM bank with its zx slice (identity matmul) and
accumulates 16 bf16 h-ktile matmuls on top. Gates use tanh-only math
(sigmoid(x) = (tanh(x/2)+1)/2) so the scalar engine never swaps
activation tables except for the per-step Exp. The h slice is cast to
bf16, transposed on DVE (32x32 StreamTranspose), and AllGathered
together with the per-shard exp-sum so every core has the full
transposed h for the next step's matmul and the global softmax
denominator for its own output slice.
"""

import numpy as np

VOCAB = 2048
EMB = 256
MAXLEN = 8
BATCH = 16
NCORES = 8
U = VOCAB // NCORES          # 256 hidden units per core
AGW = 257 * 16               # allgather payload elems per core (4112)

_CACHE = {}


def _host_tokens(input_point):
    """token[b,t] = argmax_k((k/V <= v) & (v <= (k+1)/V)), first-true wins.
    Exact: v is fp32, k/V dyadic; replicate in float64."""
    v = input_point[:, :MAXLEN].astype(np.float64)
    u = v * VOCAB
    j = np.floor(u)
    exact = (u == j) & (j > 0)
    tok = np.where(exact, j - 1, j)
    return np.clip(tok, 0, VOCAB - 1).astype(np.int32)


def _build_nc(masked, b_nonzero):
    import concourse.bass as bass
    import concourse.mybir as mybir
    import concourse.tile as tile
    from concourse import bacc
    from concourse.masks import make_identity

    f32 = mybir.dt.float32
    bf16 = mybir.dt.bfloat16
    AF = mybir.ActivationFunctionType
    OP = mybir.AluOpType

    nc = bacc.Bacc(None, target_bir_lowering=False, debug=False)

    wblk = nc.dram_tensor("wblk", [2305, 4 * U], bf16, kind="ExternalInput")
    xt_ext = nc.dram_tensor("xt", [128, 2 * MAXLEN * BATCH], bf16, kind="ExternalInput")
    mask_ext = nc.dram_tensor("masks", [BATCH, MAXLEN], f32, kind="ExternalInput")
    out_ext = nc.dram_tensor("out", [BATCH, MAXLEN * U], f32, kind="ExternalOutput")

    rg = [list(range(NCORES))]

    with tile.TileContext(nc) as tc:
        with (
            tc.tile_pool(name="const", bufs=1) as cp,
            tc.tile_pool(name="state", bufs=2) as sp,
            tc.tile_pool(name="work", bufs=3) as wk,
            tc.tile_pool(name="zps", bufs=2, space="PSUM") as psz,
            tc.tile_pool(name="zxps", bufs=1, space="PSUM") as psx,
            tc.tile_pool(name="dram", bufs=3, space="DRAM") as dp,
        ):
            identity = cp.tile([128, 128], f32)
            make_identity(nc, identity[:])
            idb = cp.tile([16, 16], bf16)
            nc.vector.tensor_copy(idb[:], identity[0:16, 0:16])

            xt_sb = cp.tile([128, 2 * MAXLEN * BATCH], bf16)
            nc.sync.dma_start(xt_sb[:], xt_ext[:])
            mask_sb = cp.tile([BATCH, MAXLEN], f32)
            nc.sync.dma_start(mask_sb[:], mask_ext[:])

            wsb = cp.tile([128, 18 * 1024], bf16)
            # x-tiles first (zx precompute unblocks), then h-tiles
            for j in (16, 17):
                nc.sync.dma_start(
                    wsb[:, 1024 * j:1024 * (j + 1)], wblk[128 * j:128 * (j + 1), :]
                )
            if b_nonzero:
                wb = cp.tile([1, 1024], bf16)
                onesrow = cp.tile([1, 128], bf16)
                nc.gpsimd.memset(onesrow[:], 1.0)
                nc.sync.dma_start(wb[:], wblk[2304:2305, :])
            for j in range(16):
                nc.sync.dma_start(
                    wsb[:, 1024 * j:1024 * (j + 1)], wblk[128 * j:128 * (j + 1), :]
                )

            # ---- zx = x @ Wk (+ b) for all steps at once: (128=(t,b), 1024) ----
            zx_ps0 = psx.tile([128, 512], f32, tag="zx0")
            zx_ps1 = psx.tile([128, 512], f32, tag="zx1")
            zx_ps = [zx_ps0, zx_ps1]
            for i, cb in ((0, 0), (1, 512)):
                nc.tensor.matmul(
                    zx_ps[i][:], xt_sb[:, 0:128],
                    wsb[:, 1024 * 16 + cb:1024 * 16 + cb + 512],
                    start=True, stop=False,
                )
                nc.tensor.matmul(
                    zx_ps[i][:], xt_sb[:, 128:256],
                    wsb[:, 1024 * 17 + cb:1024 * 17 + cb + 512],
                    start=False, stop=not b_nonzero,
                )
                if b_nonzero:
                    nc.tensor.matmul(
                        zx_ps[i][:], onesrow[:], wb[:, cb:cb + 512],
                        start=False, stop=True,
                    )
            zx_sb = cp.tile([128, 1024], bf16)
            nc.vector.tensor_copy(zx_sb[:, 0:512], zx_ps[0][:])
            nc.vector.tensor_copy(zx_sb[:, 512:1024], zx_ps[1][:])
            # rearrange to (16=batch, 8 steps x 1024)
            zx_steps = cp.tile([BATCH, MAXLEN * 1024], bf16)
            for t in range(MAXLEN):
                nc.sync.dma_start(
                    zx_steps[:, 1024 * t:1024 * (t + 1)],
                    zx_sb[16 * t:16 * (t + 1), :],
                )

            exp_sb = cp.tile([BATCH, MAXLEN * U], f32)
            out_sb = cp.tile([BATCH, MAXLEN * U], f32)

            # bf16 h staging for transpose: rows 16:32 zeroed once per slot
            hb_slots = []
            for i in range(2):
                hb = cp.tile([32, U], bf16, tag=f"hb{i}")
                nc.vector.memset(hb[:], 0.0)
                hb_slots.append(hb)

            if masked:
                h_prev = sp.tile([BATCH, U], f32, tag="h")
                nc.vector.memset(h_prev[:], 0.0)
            c_prev = sp.tile([BATCH, U], f32, tag="c")
            nc.vector.memset(c_prev[:], 0.0)
            hT_full = None

            for t in range(MAXLEN):
                # ---- z for step t in 2 PSUM banks (or zx only at t=0) ----
                if t == 0:
                    z0 = zx_steps[:, 0:512]
                    z1 = zx_steps[:, 512:1024]
                else:
                    pz0 = psz.tile([BATCH, 512], f32, tag="z0")
                    pz1 = psz.tile([BATCH, 512], f32, tag="z1")
                    for pz, cb in ((pz0, 0), (pz1, 512)):
                        nc.tensor.matmul(
                            pz[:], idb[:],
                            zx_steps[:, 1024 * t + cb:1024 * t + cb + 512],
                            start=True, stop=False,
                        )
                        for j in range(16):
                            nc.tensor.matmul(
                                pz[:], hT_full[:, 16 * j:16 * j + 16],
                                wsb[:, 1024 * j + cb:1024 * j + cb + 512],
                                start=False, stop=(j == 15),
                            )
                    z0, z1 = pz0[:], pz1[:]

                # ---- gates, tanh-only: sig(x) = (tanh(x/2)+1)/2 ----
                # z layout: [i (0:256) | f (256:512)] in z0, [g | o] in z1
                t_if = wk.tile([BATCH, 2 * U], f32, tag="tif")
                tg = wk.tile([BATCH, U], f32, tag="tg")
                t_o = wk.tile([BATCH, U], f32, tag="to")
                nc.scalar.activation(t_if[:], z0, AF.Tanh, scale=0.5)
                nc.scalar.activation(tg[:], z1[:, 0:U], AF.Tanh)
                nc.scalar.activation(t_o[:], z1[:, U:2 * U], AF.Tanh, scale=0.5)

                # 2*c_new = (t_f+1)*c + (t_i+1)*tg  (since 2*sig(x)=tanh(x/2)+1)
                t1 = wk.tile([BATCH, U], f32, tag="t1")
                t2 = wk.tile([BATCH, U], f32, tag="t2")
                s = wk.tile([BATCH, U], f32, tag="s")
                nc.vector.scalar_tensor_tensor(
                    t2[:], t_if[:, 0:U], 1.0, tg[:], OP.add, OP.mult
                )
                nc.vector.scalar_tensor_tensor(
                    t1[:], t_if[:, U:2 * U], 1.0, c_prev[:], OP.add, OP.mult
                )
                nc.vector.tensor_tensor(s[:], t1[:], t2[:], OP.add)

                hbt = hb_slots[t % 2]
                acc = wk.tile([BATCH, 1], f32, tag="acc")
                c_next = sp.tile([BATCH, U], f32, tag="c")
                if not masked:
                    # c_next = s/2 ; tanh(c_next) = tanh(s * 0.5)
                    nc.vector.tensor_scalar_mul(c_next[:], s[:], 0.5)
                    tc_ = wk.tile([BATCH, U], f32, tag="tc")
                    nc.scalar.activation(tc_[:], s[:], AF.Tanh, scale=0.5)
                    # 2h = (t_o+1)*tanh(c)
                    hn2 = wk.tile([BATCH, U], f32, tag="hn2")
                    nc.vector.scalar_tensor_tensor(
                        hn2[:], t_o[:], 1.0, tc_[:], OP.add, OP.mult
                    )
                    # bf16 h for the gather; exp(h) = exp(hn2 * 0.5)
                    nc.vector.tensor_scalar_mul(hbt[0:16, :], hn2[:], 0.5)
                    nc.scalar.activation(
                        exp_sb[:, U * t:U * (t + 1)], hn2[:], AF.Exp,
                        scale=0.5, accum_out=acc[:],
                    )
                else:
                    m_t = mask_sb[:, t:t + 1]
                    cn = wk.tile([BATCH, U], f32, tag="cn")
                    dm = wk.tile([BATCH, U], f32, tag="dm")
                    nc.vector.tensor_scalar_mul(cn[:], s[:], 0.5)
                    nc.vector.tensor_tensor(dm[:], cn[:], c_prev[:], OP.subtract)
                    nc.vector.scalar_tensor_tensor(
                        c_next[:], dm[:], m_t, c_prev[:], OP.mult, OP.add
                    )
                    tc_ = wk.tile([BATCH, U], f32, tag="tc")
                    nc.scalar.activation(tc_[:], c_next[:], AF.Tanh)
                    hn = wk.tile([BATCH, U], f32, tag="hn")
                    dh = wk.tile([BATCH, U], f32, tag="dh")
                    h_next = sp.tile([BATCH, U], f32, tag="h")
                    nc.vector.scalar_tensor_tensor(
                        hn[:], t_o[:], 1.0, tc_[:], OP.add, OP.mult
                    )
                    nc.vector.tensor_scalar_mul(hn[:], hn[:], 0.5)
                    nc.vector.tensor_tensor(dh[:], hn[:], h_prev[:], OP.subtract)
                    nc.vector.scalar_tensor_tensor(
                        h_next[:], dh[:], m_t, h_prev[:], OP.mult, OP.add
                    )
                    nc.vector.tensor_copy(hbt[0:16, :], h_next[:])
                    nc.scalar.activation(
                        exp_sb[:, U * t:U * (t + 1)], h_next[:], AF.Exp,
                        accum_out=acc[:],
                    )
                    h_prev = h_next

                accb = wk.tile([BATCH, 1], bf16, tag="accb")
                nc.vector.tensor_copy(accb[:], acc[:])

                # ---- transpose h slice on DVE: (32,256) -> 32x32 blocks ----
                trT = wk.tile([32, U], bf16, tag="trT")
                nc.vector.transpose(trT[:], hbt[:])

                # ---- AllGather [hT_slice (256x16) | exp-sums (16)] ----
                agin = dp.tile([257, 16], bf16, tag="agin")
                agout = dp.tile([NCORES, AGW], bf16, tag="agout")
                nc.sync.dma_start(
                    agin[0:256, :].rearrange("(j p) q -> p j q", p=32),
                    trT[:].rearrange("p (j q) -> p j q", q=32)[:, :, 0:16],
                )
                nc.gpsimd.dma_start(
                    agin[256:257, :].rearrange("a b -> b a"), accb[:]
                )
                nc.gpsimd.collective_compute(
                    "AllGather",
                    OP.bypass,
                    replica_groups=rg,
                    ins=[agin[:, :].opt()],
                    outs=[agout[:, :].opt()],
                )

                # ---- unpack gathered hT (not needed after last step) ----
                if t < MAXLEN - 1:
                    hT_full = sp.tile([128, 16 * 16], bf16, tag="hT")
                    for m, eng in ((0, nc.sync), (1, nc.scalar)):
                        eng.dma_start(
                            hT_full[:].rearrange("p (r q) -> p r q", q=32)[
                                :, :, 16 * m:16 * m + 16
                            ],
                            agout[:, 2048 * m:2048 * m + 2048].rearrange(
                                "r (p b) -> p r b", p=128, b=16
                            ),
                        )

                # ---- global denominator + own-slice softmax + out ----
                sums_t = wk.tile([BATCH, NCORES], bf16, tag="sums")
                nc.gpsimd.dma_start(
                    sums_t[:], agout[:, 4096:4112].rearrange("r b -> b r")
                )
                tot = wk.tile([BATCH, 1], f32, tag="tot")
                rec = wk.tile([BATCH, 1], f32, tag="rec")
                nc.vector.tensor_reduce(
                    tot[:], sums_t[:], mybir.AxisListType.X, OP.add
                )
                nc.vector.reciprocal(rec[:], tot[:])
                nc.vector.tensor_scalar_mul(
                    out_sb[:, U * t:U * (t + 1)], exp_sb[:, U * t:U * (t + 1)],
                    rec[:],
                )
                nc.gpsimd.dma_start(
                    out_ext[:, U * t:U * (t + 1)], out_sb[:, U * t:U * (t + 1)]
                )

                c_prev = c_next

    nc.compile()
    return nc


def _get_nc(masked=False, b_nonzero=False):
    key = (masked, b_nonzero)
    if key not in _CACHE:
        _CACHE[key] = _build_nc(masked, b_nonzero)
    return _CACHE[key]


def _host_prep(input_point, E, Wk, Wr, b):
    import ml_dtypes
    bf = ml_dtypes.bfloat16

    ip = np.ascontiguousarray(np.asarray(input_point, dtype=np.float32))
    E = np.asarray(E, dtype=np.float32)
    Wk = np.asarray(Wk, dtype=np.float32)
    Wr = np.asarray(Wr, dtype=np.float32)
    b = np.asarray(b, dtype=np.float32)

    tokens = _host_tokens(ip)                                # (B, T)
    masks = (tokens != 0).astype(np.float32)                 # (B, T)
    X = E[tokens]                                            # (B, T, EMB)

    # xt[p, 128*half + 16*t + b] = X[b, t, 128*half + p]
    xt = np.transpose(X.reshape(BATCH, MAXLEN, 2, 128), (2, 3, 1, 0))  # (2,128,T,B)
    xt = np.ascontiguousarray(
        np.transpose(xt, (1, 0, 2, 3)).reshape(128, 2 * MAXLEN * BATCH)
    ).astype(bf)

    W_aug = np.vstack([Wr, Wk, b[None, :]]).astype(np.float32)  # (2305, 4V)
    in_maps = []
    for k in range(NCORES):
        cols = np.concatenate(
            [np.arange(g * VOCAB + k * U, g * VOCAB + (k + 1) * U) for g in range(4)]
        )
        in_maps.append({
            "wblk": np.ascontiguousarray(W_aug[:, cols]).astype(bf),
            "xt": xt,
            "masks": np.ascontiguousarray(masks),
        })
    flags = (bool((masks != 1.0).any()), bool(np.any(b != 0.0)))
    return in_maps, flags


def kernel(input_point, E, Wk, Wr, b):
    from concourse.bass_utils import run_bass_kernel_spmd

    in_maps, flags = _host_prep(input_point, E, Wk, Wr, b)
    nc = _get_nc(*flags)
    res = run_bass_kernel_spmd(nc, in_maps, list(range(NCORES)))
    results = res.results

    out = np.empty((BATCH, MAXLEN, VOCAB), dtype=np.float32)
    for k in range(NCORES):
        blk = results[k]["out"].reshape(BATCH, MAXLEN, U)    # (B, T, U)
        out[:, :, k * U:(k + 1) * U] = blk
    return out


# revision 16
# speedup vs baseline: 1.5828x; 1.0212x over previous
"""Trainium2 Bass kernel for nn_DAriEL_Decoder_Cell_1_88064009437441.

Key structural fact about the reference: the decoder cell resets
`one_softmax`/`unfolding` to their initial values at every t>0 (faithful
tf.cond port), so token selection at step t uses the UNIFORM distribution
and input_point[:, t] only — tokens never depend on the LM. The LM outputs
(the actual kernel result) are softmax(h_t) of a single 8-step LSTM scan
over the decoded tokens, since the per-step prefixes are nested.

Host: exact uniform-interval token search (all quantities are dyadic
rationals, exact in fp32) + embedding gather + weight re-layout (bf16).
Device (8 cores, SPMD): gate-dim (hidden-unit) sharded LSTM, 256 units
per core. x@Wk for all 8 steps is computed once up front; per step each
core seeds a PSUM bank with its zx slice (identity matmul) and
accumulates 16 bf16 h-ktile matmuls on top. Gates use tanh-only math
(sigmoid(x) = (tanh(x/2)+1)/2) so the scalar engine never swaps
activation tables except for the per-step Exp. The h slice is cast to
bf16, transposed on DVE (32x32 StreamTranspose), and AllGathered
together with the per-shard exp-sum so every core has the full
transposed h for the next step's matmul and the global softmax
denominator for its own output slice.
"""

import numpy as np

VOCAB = 2048
EMB = 256
MAXLEN = 8
BATCH = 16
NCORES = 8
U = VOCAB // NCORES          # 256 hidden units per core
AGW = 257 * 16               # allgather payload elems per core (4112)

_CACHE = {}


def _host_tokens(input_point):
    """token[b,t] = argmax_k((k/V <= v) & (v <= (k+1)/V)), first-true wins.
    Exact: v is fp32, k/V dyadic; replicate in float64."""
    v = input_point[:, :MAXLEN].astype(np.float64)
    u = v * VOCAB
    j = np.floor(u)
    exact = (u == j) & (j > 0)
    tok = np.where(exact, j - 1, j)
    return np.clip(tok, 0, VOCAB - 1).astype(np.int32)


def _build_nc(masked, b_nonzero):
    import concourse.bass as bass
    import concourse.mybir as mybir
    import concourse.tile as tile
    from concourse import bacc
    from concourse.masks import make_identity

    f32 = mybir.dt.float32
    bf16 = mybir.dt.bfloat16
    AF = mybir.ActivationFunctionType
    OP = mybir.AluOpType

    nc = bacc.Bacc(None, target_bir_lowering=False, debug=False)

    wblk = nc.dram_tensor("wblk", [2305, 4 * U], bf16, kind="ExternalInput")
    xt_ext = nc.dram_tensor("xt", [128, 2 * MAXLEN * BATCH], bf16, kind="ExternalInput")
    mask_ext = nc.dram_tensor("masks", [BATCH, MAXLEN], f32, kind="ExternalInput")
    out_ext = nc.dram_tensor("out", [BATCH, MAXLEN * U], f32, kind="ExternalOutput")

    rg = [list(range(NCORES))]

    with tile.TileContext(nc) as tc:
        with (
            tc.tile_pool(name="const", bufs=1) as cp,
            tc.tile_pool(name="state", bufs=2) as sp,
            tc.tile_pool(name="work", bufs=3) as wk,
            tc.tile_pool(name="zps", bufs=2, space="PSUM") as psz,
            tc.tile_pool(name="zxps", bufs=1, space="PSUM") as psx,
            tc.tile_pool(name="dram", bufs=3, space="DRAM") as dp,
        ):
            identity = cp.tile([128, 128], f32)
            make_identity(nc, identity[:])
            idb = cp.tile([16, 16], bf16)
            nc.vector.tensor_copy(idb[:], identity[0:16, 0:16])

            xt_sb = cp.tile([128, 2 * MAXLEN * BATCH], bf16)
            nc.sync.dma_start(xt_sb[:], xt_ext[:])
            mask_sb = cp.tile([BATCH, MAXLEN], f32)
            nc.sync.dma_start(mask_sb[:], mask_ext[:])

            wsb = cp.tile([128, 18 * 1024], bf16)
            # x-tiles first (zx precompute unblocks), then h-tiles;
            # round-robin issue engines so descriptor gen parallelizes
            dma_engs = [nc.sync, nc.scalar, nc.gpsimd]
            for i, j in enumerate((16, 17)):
                dma_engs[i % 3].dma_start(
                    wsb[:, 1024 * j:1024 * (j + 1)], wblk[128 * j:128 * (j + 1), :]
                )
            if b_nonzero:
                wb = cp.tile([1, 1024], bf16)
                onesrow = cp.tile([1, 128], bf16)
                nc.gpsimd.memset(onesrow[:], 1.0)
                nc.sync.dma_start(wb[:], wblk[2304:2305, :])
            for j in range(16):
                dma_engs[j % 3].dma_start(
                    wsb[:, 1024 * j:1024 * (j + 1)], wblk[128 * j:128 * (j + 1), :]
                )

            # ---- zx = x @ Wk (+ b) for all steps at once: (128=(t,b), 1024) ----
            zx_ps0 = psx.tile([128, 512], f32, tag="zx0")
            zx_ps1 = psx.tile([128, 512], f32, tag="zx1")
            zx_ps = [zx_ps0, zx_ps1]
            for i, cb in ((0, 0), (1, 512)):
                nc.tensor.matmul(
                    zx_ps[i][:], xt_sb[:, 0:128],
                    wsb[:, 1024 * 16 + cb:1024 * 16 + cb + 512],
                    start=True, stop=False,
                )
                nc.tensor.matmul(
                    zx_ps[i][:], xt_sb[:, 128:256],
                    wsb[:, 1024 * 17 + cb:1024 * 17 + cb + 512],
                    start=False, stop=not b_nonzero,
                )
                if b_nonzero:
                    nc.tensor.matmul(
                        zx_ps[i][:], onesrow[:], wb[:, cb:cb + 512],
                        start=False, stop=True,
                    )
            zx_sb = cp.tile([128, 1024], bf16)
            nc.vector.tensor_copy(zx_sb[:, 0:512], zx_ps[0][:])
            nc.vector.tensor_copy(zx_sb[:, 512:1024], zx_ps[1][:])
            # rearrange to (16=batch, 8 steps x 1024)
            zx_steps = cp.tile([BATCH, MAXLEN * 1024], bf16)
            for t in range(MAXLEN):
                nc.sync.dma_start(
                    zx_steps[:, 1024 * t:1024 * (t + 1)],
                    zx_sb[16 * t:16 * (t + 1), :],
                )

            exp_sb = cp.tile([BATCH, MAXLEN * U], f32)
            out_sb = cp.tile([BATCH, MAXLEN * U], f32)

            # bf16 h staging for transpose: rows 16:32 zeroed once per slot
            hb_slots = []
            for i in range(2):
                hb = cp.tile([32, U], bf16, tag=f"hb{i}")
                nc.vector.memset(hb[:], 0.0)
                hb_slots.append(hb)

            if masked:
                h_prev = sp.tile([BATCH, U], f32, tag="h")
                nc.vector.memset(h_prev[:], 0.0)
            c_prev = sp.tile([BATCH, U], f32, tag="c")
            nc.vector.memset(c_prev[:], 0.0)
            hT_full = None

            for t in range(MAXLEN):
                # ---- z for step t in 2 PSUM banks (or zx only at t=0) ----
                if t == 0:
                    z0 = zx_steps[:, 0:512]
                    z1 = zx_steps[:, 512:1024]
                else:
                    pz0 = psz.tile([BATCH, 512], f32, tag="z0")
                    pz1 = psz.tile([BATCH, 512], f32, tag="z1")
                    for pz, cb in ((pz0, 0), (pz1, 512)):
                        nc.tensor.matmul(
                            pz[:], idb[:],
                            zx_steps[:, 1024 * t + cb:1024 * t + cb + 512],
                            start=True, stop=False,
                        )
                        for j in range(16):
                            nc.tensor.matmul(
                                pz[:], hT_full[:, 16 * j:16 * j + 16],
                                wsb[:, 1024 * j + cb:1024 * j + cb + 512],
                                start=False, stop=(j == 15),
                            )
                    z0, z1 = pz0[:], pz1[:]

                # ---- gates, tanh-only: sig(x) = (tanh(x/2)+1)/2 ----
                # z layout: [i (0:256) | f (256:512)] in z0, [g | o] in z1
                t_if = wk.tile([BATCH, 2 * U], f32, tag="tif")
                tg = wk.tile([BATCH, U], f32, tag="tg")
                t_o = wk.tile([BATCH, U], f32, tag="to")
                nc.scalar.activation(t_if[:], z0, AF.Tanh, scale=0.5)
                nc.scalar.activation(tg[:], z1[:, 0:U], AF.Tanh)
                nc.scalar.activation(t_o[:], z1[:, U:2 * U], AF.Tanh, scale=0.5)

                hbt = hb_slots[t % 2]
                acc = wk.tile([BATCH, 1], f32, tag="acc")
                c_next = sp.tile([BATCH, U], f32, tag="c")
                if not masked:
                    # State C = 2c, hb = 2h; Wr is pre-halved on the host so
                    # z += (2h) @ (Wr/2). 2*sig(x) = tanh(x/2)+1.
                    # C_new = 0.5*(t_f+1)*C + (t_i+1)*tg
                    t1 = wk.tile([BATCH, U], f32, tag="t1")
                    t2 = wk.tile([BATCH, U], f32, tag="t2")
                    nc.vector.scalar_tensor_tensor(
                        t2[:], t_if[:, 0:U], 1.0, tg[:], OP.add, OP.mult
                    )
                    nc.vector.scalar_tensor_tensor(
                        t1[:], t_if[:, U:2 * U], 1.0, c_prev[:], OP.add, OP.mult
                    )
                    nc.vector.scalar_tensor_tensor(
                        c_next[:], t1[:], 0.5, t2[:], OP.mult, OP.add
                    )
                    tc_ = wk.tile([BATCH, U], f32, tag="tc")
                    nc.scalar.activation(tc_[:], c_next[:], AF.Tanh, scale=0.5)
                    # hb = 2h = (t_o+1)*tanh(c), written bf16 directly
                    nc.vector.scalar_tensor_tensor(
                        hbt[0:16, :], t_o[:], 1.0, tc_[:], OP.add, OP.mult
                    )
                    # exp(h) = exp(hb * 0.5), numerator + per-shard sum
                    nc.scalar.activation(
                        exp_sb[:, U * t:U * (t + 1)], hbt[0:16, :], AF.Exp,
                        scale=0.5, accum_out=acc[:],
                    )
                else:
                    # Same doubled representation (C=2c, H=2h); mask blending
                    # is linear so it works unchanged on doubled state.
                    m_t = mask_sb[:, t:t + 1]
                    t1 = wk.tile([BATCH, U], f32, tag="t1")
                    t2 = wk.tile([BATCH, U], f32, tag="t2")
                    cn = wk.tile([BATCH, U], f32, tag="cn")
                    dm = wk.tile([BATCH, U], f32, tag="dm")
                    nc.vector.scalar_tensor_tensor(
                        t2[:], t_if[:, 0:U], 1.0, tg[:], OP.add, OP.mult
                    )
                    nc.vector.scalar_tensor_tensor(
                        t1[:], t_if[:, U:2 * U], 1.0, c_prev[:], OP.add, OP.mult
                    )
                    nc.vector.scalar_tensor_tensor(
                        cn[:], t1[:], 0.5, t2[:], OP.mult, OP.add
                    )
                    nc.vector.tensor_tensor(dm[:], cn[:], c_prev[:], OP.subtract)
                    nc.vector.scalar_tensor_tensor(
                        c_next[:], dm[:], m_t, c_prev[:], OP.mult, OP.add
                    )
                    tc_ = wk.tile([BATCH, U], f32, tag="tc")
                    nc.scalar.activation(tc_[:], c_next[:], AF.Tanh, scale=0.5)
                    hn = wk.tile([BATCH, U], f32, tag="hn")
                    dh = wk.tile([BATCH, U], f32, tag="dh")
                    h_next = sp.tile([BATCH, U], f32, tag="h")
                    nc.vector.scalar_tensor_tensor(
                        hn[:], t_o[:], 1.0, tc_[:], OP.add, OP.mult
                    )
                    nc.vector.tensor_tensor(dh[:], hn[:], h_prev[:], OP.subtract)
                    nc.vector.scalar_tensor_tensor(
                        h_next[:], dh[:], m_t, h_prev[:], OP.mult, OP.add
                    )
                    nc.vector.tensor_copy(hbt[0:16, :], h_next[:])
                    nc.scalar.activation(
                        exp_sb[:, U * t:U * (t + 1)], h_next[:], AF.Exp,
                        scale=0.5, accum_out=acc[:],
                    )
                    h_prev = h_next

                accb = wk.tile([BATCH, 1], bf16, tag="accb")
                nc.vector.tensor_copy(accb[:], acc[:])

                # ---- transpose h slice on DVE: (32,256) -> 32x32 blocks ----
                trT = wk.tile([32, U], bf16, tag="trT")
                nc.vector.transpose(trT[:], hbt[:])

                # ---- AllGather [hT image (128x32) | exp-sums (16)] ----
                # agin flat layout: elem(32p + 16m + b) = hT[128m+p, b] so the
                # gathered block unpacks with 64B-contiguous runs per partition
                agin = dp.tile([AGW], bf16, tag="agin")
                agout = dp.tile([NCORES, AGW], bf16, tag="agout")
                trT_v = trT[:].rearrange("p (m ph q) -> p m ph q", m=2, ph=4, q=32)
                agin_v = agin[0:4096].rearrange(
                    "(ph pl m b) -> pl ph m b", ph=4, pl=32, m=2, b=16
                )
                nc.sync.dma_start(
                    agin_v[:, :, 0:1, :], trT_v[:, 0:1, :, 0:16]
                )
                nc.scalar.dma_start(
                    agin_v[:, :, 1:2, :], trT_v[:, 1:2, :, 0:16]
                )
                nc.gpsimd.dma_start(agin[4096:4112], accb[:, 0])
                nc.gpsimd.collective_compute(
                    "AllGather",
                    OP.bypass,
                    replica_groups=rg,
                    ins=[agin[:].opt()],
                    outs=[agout[:, :].opt()],
                )

                # ---- unpack gathered hT (not needed after last step) ----
                if t < MAXLEN - 1:
                    hT_full = sp.tile([128, 16 * 16], bf16, tag="hT")
                    nc.sync.dma_start(
                        hT_full[:].rearrange("p (r f) -> p r f", r=8, f=32),
                        agout[:, 0:4096].rearrange("r (p f) -> p r f", p=128, f=32),
                    )

                # ---- global denominator + own-slice softmax + out ----
                sums_t = wk.tile([BATCH, NCORES], bf16, tag="sums")
                nc.gpsimd.dma_start(
                    sums_t[:], agout[:, 4096:4112].rearrange("r b -> b r")
                )
                tot = wk.tile([BATCH, 1], f32, tag="tot")
                rec = wk.tile([BATCH, 1], f32, tag="rec")
                nc.vector.tensor_reduce(
                    tot[:], sums_t[:], mybir.AxisListType.X, OP.add
                )
                nc.vector.reciprocal(rec[:], tot[:])
                nc.vector.tensor_scalar_mul(
                    out_sb[:, U * t:U * (t + 1)], exp_sb[:, U * t:U * (t + 1)],
                    rec[:],
                )
                nc.gpsimd.dma_start(
                    out_ext[:, U * t:U * (t + 1)], out_sb[:, U * t:U * (t + 1)]
                )

                c_prev = c_next

    nc.compile()
    return nc


def _get_nc(masked=False, b_nonzero=False):
    key = (masked, b_nonzero)
    if key not in _CACHE:
        _CACHE[key] = _build_nc(masked, b_nonzero)
    return _CACHE[key]


def _host_prep(input_point, E, Wk, Wr, b):
    import ml_dtypes
    bf = ml_dtypes.bfloat16

    ip = np.ascontiguousarray(np.asarray(input_point, dtype=np.float32))
    E = np.asarray(E, dtype=np.float32)
    Wk = np.asarray(Wk, dtype=np.float32)
    Wr = np.asarray(Wr, dtype=np.float32)
    b = np.asarray(b, dtype=np.float32)

    tokens = _host_tokens(ip)                                # (B, T)
    masks = (tokens != 0).astype(np.float32)                 # (B, T)
    X = E[tokens]                                            # (B, T, EMB)

    # xt[p, 128*half + 16*t + b] = X[b, t, 128*half + p]
    xt = np.transpose(X.reshape(BATCH, MAXLEN, 2, 128), (2, 3, 1, 0))  # (2,128,T,B)
    xt = np.ascontiguousarray(
        np.transpose(xt, (1, 0, 2, 3)).reshape(128, 2 * MAXLEN * BATCH)
    ).astype(bf)

    # Wr halved: the device carries h in doubled representation (hb = 2h)
    W_aug = np.vstack([Wr * 0.5, Wk, b[None, :]]).astype(np.float32)  # (2305, 4V)
    in_maps = []
    for k in range(NCORES):
        cols = np.concatenate(
            [np.arange(g * VOCAB + k * U, g * VOCAB + (k + 1) * U) for g in range(4)]
        )
        in_maps.append({
            "wblk": np.ascontiguousarray(W_aug[:, cols]).astype(bf),
            "xt": xt,
            "masks": np.ascontiguousarray(masks),
        })
    flags = (bool((masks != 1.0).any()), bool(np.any(b != 0.0)))
    return in_maps, flags


def kernel(input_point, E, Wk, Wr, b):
    from concourse.bass_utils import run_bass_kernel_spmd

    in_maps, flags = _host_prep(input_point, E, Wk, Wr, b)
    nc = _get_nc(*flags)
    res = run_bass_kernel_spmd(nc, in_maps, list(range(NCORES)))
    results = res.results

    out = np.empty((BATCH, MAXLEN, VOCAB), dtype=np.float32)
    for k in range(NCORES):
        blk = results[k]["out"].reshape(BATCH, MAXLEN, U)    # (B, T, U)
        out[:, :, k * U:(k + 1) * U] = blk
    return out
